# revision 1
# baseline (speedup 1.0000x reference)
"""MoE (16 routed experts, top-2, + shared expert) on 8 Trainium2 cores.

Strategy (expert-parallel, host-side dispatch):
  - Host computes the gate (softmax + top-2) and gathers each expert's
    tokens; core c owns experts 2c and 2c+1.
  - The shared expert is data-parallel: core c processes tokens
    [c*T/8, (c+1)*T/8).
  - Each core runs the same program: three gated-MLP "blocks"
    (expert0, expert1, shared) in a transposed layout
        zT = W2^T @ (u * silu(g)),  [u;g]^T = W1^T @ xT
    so no on-chip transposes are needed anywhere.
  - Host pre-arranges weights per-core into the exact SBUF tile layout so
    every DMA reads 4KB-contiguous runs per partition (full HBM rate).
  - Host applies the top-2 combine weights and scatter-adds expert
    outputs, then adds the shared-expert outputs.

Matmuls run as fp32r (full PE rate at free-dim >= 256, ~3e-4 rel err).
"""

import sys

for _p in ("/opt/trn_rl_repo", "/root/.axon_site/_ro/trn_rl_repo"):
    if _p not in sys.path:
        sys.path.insert(0, _p)

import contextlib
import os

import numpy as np

import concourse.bass as bass  # noqa: F401
import concourse.tile as tile
from concourse import bacc, mybir
from concourse.bass_utils import run_bass_kernel_spmd

try:  # tracing needs the axon NTFF hook; absent in some containers
    from antenv import axon_hooks as _axon_hooks  # noqa: F401
except ImportError:
    os.environ.setdefault("BASS_NEVER_TRACE", "1")

B, S, D = 2, 1024, 1024
H = 512           # routed expert hidden
HS = 1024         # shared expert hidden
E = 16
ROUTE_SCALE = 1.0
T = B * S
N_CORES = 8
EPC = E // N_CORES          # experts per core
TDP = T // N_CORES          # shared-expert tokens per core
P = 128
KD = D // P                 # fc1 contraction chunks
MD = D // P                 # fc2 output chunks

DTYPE = "f16"          # "f32r" | "bf16" | "f16"
F32 = mybir.dt.float32
if DTYPE == "f32r":
    FPR, NPT = mybir.dt.float32r, np.float32
elif DTYPE == "bf16":
    import ml_dtypes
    FPR, NPT = mybir.dt.bfloat16, ml_dtypes.bfloat16
else:
    FPR, NPT = mybir.dt.float16, np.float16
ACT = mybir.ActivationFunctionType

LAST_RESULTS = None
_NC_CACHE = {}


def _build_nc(CR, reps=1):
    """SPMD program: two routed-expert blocks (capacity CR) + shared block.

    Weight params arrive pre-arranged:
      w1: [2*NH, P, KD*P]   chunk j = hc*2 + (0:u | 1:g), 4KB runs
      w2: [MD, P, NH*P]
      x:  [P, KD*C]
    reps > 1 wraps the body in a dynamic loop (benchmarking only)."""
    nc = bacc.Bacc(None, target_bir_lowering=False)

    blocks = []
    for i in range(EPC):
        NH = H // P
        blocks.append((
            nc.declare_dram_parameter(f"w1e{i}", [2 * NH, P, KD * P], FPR, isOutput=False),
            nc.declare_dram_parameter(f"w2e{i}", [MD, P, NH * P], FPR, isOutput=False),
            H,
            nc.declare_dram_parameter(f"xg{i}", [P, KD * CR], FPR, isOutput=False),
            CR,
            nc.declare_dram_parameter(f"zg{i}", [D, CR], F32, isOutput=True),
        ))
    NHS = HS // P
    blocks.append((
        nc.declare_dram_parameter("ws1", [2 * NHS, P, KD * P], FPR, isOutput=False),
        nc.declare_dram_parameter("ws2", [MD, P, NHS * P], FPR, isOutput=False),
        HS,
        nc.declare_dram_parameter("xd", [P, KD * TDP], FPR, isOutput=False),
        TDP,
        nc.declare_dram_parameter("zs", [D, TDP], F32, isOutput=True),
    ))

    with tile.TileContext(nc) as tc:
        with (
            tc.tile_pool(name="xpool", bufs=2) as xpool,
            tc.tile_pool(name="w1pool", bufs=8) as w1pool,
            tc.tile_pool(name="w2pool", bufs=6) as w2pool,
            tc.tile_pool(name="hpool", bufs=2) as hpool,
            tc.tile_pool(name="spool", bufs=4) as spool,
            tc.tile_pool(name="opool", bufs=4) as opool,
            tc.tile_pool(name="psu", bufs=3, space="PSUM") as psu,
            tc.tile_pool(name="psg", bufs=3, space="PSUM") as psg,
            tc.tile_pool(name="psz", bufs=2, space="PSUM") as psz,
        ):
            pools = (xpool, w1pool, w2pool, hpool, spool, opool, psu, psg, psz)
            loop_cm = tc.For_i(0, reps, 1) if reps > 1 else contextlib.nullcontext()
            with loop_cm:
                _emit_body(nc, blocks, pools)
    nc.finalize()
    return nc


def _emit_body(nc, blocks, pools):
    xpool, w1pool, w2pool, hpool, spool, opool, psu, psg, psz = pools
    for w1, w2, HB, xt, C, zt in blocks:
        NH = HB // P
        w1_a = w1.ap()
        w2_a = w2.ap()
        xt_a = xt.ap().rearrange("p (ko c) -> p ko c", ko=KD)
        zt_a = zt.ap().rearrange("(mo mi) c -> mi mo c", mi=P)

        x_tile = xpool.tile([P, KD, C], FPR, tag=f"x{C}")
        nc.sync.dma_start(x_tile[:], xt_a)
        h_tile = hpool.tile([P, NH, C], FPR, tag=f"h{NH}_{C}")

        for hc in range(NH):
            w1t = w1pool.tile([P, 2, KD, P], FPR, tag="w1")
            nc.sync.dma_start(
                w1t[:],
                w1_a[2 * hc:2 * hc + 2].rearrange("s p (ko f) -> p s ko f", ko=KD))
            ps_u = psu.tile([P, C], F32, tag="psu")
            ps_g = psg.tile([P, C], F32, tag="psg")
            for k in range(KD):
                nc.tensor.matmul(ps_u[:], w1t[:, 0, k], x_tile[:, k],
                                 start=(k == 0), stop=(k == KD - 1))
            for k in range(KD):
                nc.tensor.matmul(ps_g[:], w1t[:, 1, k], x_tile[:, k],
                                 start=(k == 0), stop=(k == KD - 1))
            sil = spool.tile([P, C], F32, tag="sil")
            nc.scalar.activation(sil[:], ps_g[:], ACT.Silu)
            nc.vector.tensor_mul(h_tile[:, hc], ps_u[:], sil[:])

        for dp in range(MD // 2):
            w2t = w2pool.tile([P, 2, NH, P], FPR, tag=f"w2{NH}")
            nc.scalar.dma_start(
                w2t[:],
                w2_a[2 * dp:2 * dp + 2].rearrange("s p (ko f) -> p s ko f", ko=NH))
            for s2 in range(2):
                ps_z = psz.tile([P, C], F32, tag="psz")
                for k in range(NH):
                    nc.tensor.matmul(ps_z[:], w2t[:, s2, k], h_tile[:, k],
                                     start=(k == 0), stop=(k == NH - 1))
                o_tile = opool.tile([P, C], F32, tag="o")
                nc.vector.tensor_copy(o_tile[:], ps_z[:])
                nc.scalar.dma_start(zt_a[:, 2 * dp + s2], o_tile[:])


def _route(xf, Wg):
    """Host gate: softmax over expert logits, top-2 (ties -> lower index,
    matching jax.lax.top_k)."""
    logits = xf @ Wg.T
    m = logits.max(axis=-1, keepdims=True)
    p = np.exp(logits - m)
    scores = p / p.sum(axis=-1, keepdims=True)
    i1 = scores.argmax(axis=-1)
    rows = np.arange(T)
    s1 = scores[rows, i1]
    masked = scores.copy()
    masked[rows, i1] = -np.inf
    i2 = masked.argmax(axis=-1)
    s2 = scores[rows, i2]
    return i1, s1 * ROUTE_SCALE, i2, s2 * ROUTE_SCALE


def _pack_w1(W1b, HB):
    """[D, 2*HB] -> [2*NH, P, KD*P], chunk j = hc*2 + half, contiguous runs."""
    NH = HB // P
    Ar = W1b.reshape(KD, P, 2, NH, P)              # [ko, ki, half, hc, f]
    return np.ascontiguousarray(
        Ar.transpose(3, 2, 1, 0, 4).reshape(2 * NH, P, KD * P).astype(NPT))


def _pack_w2(W2b, HB):
    """[HB, D] -> [MD, P, NH*P]."""
    NH = HB // P
    Br = W2b.reshape(NH, P, MD, P)                 # [ko, ki, dc, f]
    return np.ascontiguousarray(
        Br.transpose(2, 1, 0, 3).reshape(MD, P, NH * P).astype(NPT))


def _pack_x(xTb, C):
    """[D, C] -> [P, KD*C] (zero-pads the token dim to C)."""
    n = xTb.shape[1]
    out = np.zeros((P, KD * C), dtype=NPT)
    out.reshape(P, KD, C)[:, :, :n] = xTb.reshape(KD, P, n).transpose(1, 0, 2)
    return out


def prepare(x, Wg, W1, W2, Ws1, Ws2):
    """Host routing + per-core input maps. Returns (in_maps, toks, wts, CR)."""
    x = np.asarray(x, dtype=np.float32)
    Wg = np.asarray(Wg, dtype=np.float32)
    W1 = np.asarray(W1, dtype=np.float32)
    W2 = np.asarray(W2, dtype=np.float32)
    Ws1 = np.asarray(Ws1, dtype=np.float32)
    Ws2 = np.asarray(Ws2, dtype=np.float32)

    xf = np.ascontiguousarray(x.reshape(T, D))
    i1, s1, i2, s2 = _route(xf, Wg)

    toks, wts = [], []
    for e in range(E):
        sel = np.where((i1 == e) | (i2 == e))[0]
        toks.append(sel)
        wts.append(np.where(i1[sel] == e, s1[sel], s2[sel]).astype(np.float32))

    max_n = max(len(t) for t in toks)
    CR = max(256, -(-max_n // 32) * 32)

    ws1p = _pack_w1(Ws1, HS)
    ws2p = _pack_w2(Ws2, HS)
    in_maps = []
    for c in range(N_CORES):
        im = {"ws1": ws1p, "ws2": ws2p,
              "xd": _pack_x(np.ascontiguousarray(xf[c * TDP:(c + 1) * TDP].T), TDP)}
        for i in range(EPC):
            e = EPC * c + i
            im[f"w1e{i}"] = _pack_w1(W1[e], H)
            im[f"w2e{i}"] = _pack_w2(W2[e], H)
            im[f"xg{i}"] = _pack_x(xf[toks[e]].T, CR)
        in_maps.append(im)
    return in_maps, toks, wts, CR


def kernel(x, Wg, W1, W2, Ws1, Ws2):
    global LAST_RESULTS
    in_maps, toks, wts, CR = prepare(x, Wg, W1, W2, Ws1, Ws2)

    if CR not in _NC_CACHE:
        _NC_CACHE[CR] = _build_nc(CR)
    nc = _NC_CACHE[CR]

    try:
        LAST_RESULTS = run_bass_kernel_spmd(nc, in_maps, list(range(N_CORES)))
    except Exception:
        # transient NRT device errors have been observed; retry once
        LAST_RESULTS = run_bass_kernel_spmd(nc, in_maps, list(range(N_CORES)))
    res = LAST_RESULTS.results

    out = np.zeros((T, D), dtype=np.float32)
    for c in range(N_CORES):
        for i in range(EPC):
            e = EPC * c + i
            n = len(toks[e])
            out[toks[e]] += wts[e][:, None] * res[c][f"zg{i}"][:, :n].T
        out[c * TDP:(c + 1) * TDP] += res[c]["zs"].T
    return out.reshape(B, S, D)



# revision 2
# speedup vs baseline: 1.0695x; 1.0695x over previous
"""MoE (16 routed experts, top-2, + shared expert) on 8 Trainium2 cores.

Strategy (expert-parallel, host-side dispatch):
  - Host computes the gate (softmax + top-2) and gathers each expert's
    tokens; experts are permuted so each core owns one "big" and one
    "small" expert (slot capacities CA >= CB), minimizing pad columns.
  - The shared expert is data-parallel: core c processes tokens
    [c*T/8, (c+1)*T/8).
  - Each core runs the same program: three gated-MLP blocks
    (expertA, expertB, shared) in a transposed layout
        zT = W2^T @ (u * silu(g)),  [u;g]^T = W1^T @ xT
    so no on-chip transposes are needed anywhere.
  - All weights are SBUF-resident (~96KB/partition); they stream in as
    a few ~0.5-2MB DMAs on the sync (SP) HWDGE FIFO in exactly the
    order compute consumes them, so the PE chases the DMA front with
    no queue interleaving. Outputs are staged in SBUF as f16 and
    stored with one DMA per block on the scalar (ACT) FIFO.
  - Host applies the top-2 combine weights and scatter-adds expert
    outputs, then adds the shared-expert outputs.

Matmuls run in f16 (~5e-4 rel err vs the f32 reference).
"""

import sys

for _p in ("/opt/trn_rl_repo", "/root/.axon_site/_ro/trn_rl_repo"):
    if _p not in sys.path:
        sys.path.insert(0, _p)

import contextlib
import os

import numpy as np

import concourse.bass as bass  # noqa: F401
import concourse.tile as tile
from concourse import bacc, mybir
from concourse.bass_utils import run_bass_kernel_spmd

try:  # tracing needs the axon NTFF hook; absent in some containers
    from antenv import axon_hooks as _axon_hooks  # noqa: F401
except ImportError:
    os.environ.setdefault("BASS_NEVER_TRACE", "1")

B, S, D = 2, 1024, 1024
H = 512           # routed expert hidden
HS = 1024         # shared expert hidden
E = 16
ROUTE_SCALE = 1.0
T = B * S
N_CORES = 8
EPC = E // N_CORES          # experts per core
TDP = T // N_CORES          # shared-expert tokens per core
P = 128
KD = D // P                 # fc1 contraction chunks
MD = D // P                 # fc2 output chunks
NH = H // P                 # routed fc1 output chunks (per half)
NHS = HS // P               # shared fc1 output chunks (per half)
GH = 2                      # fc1 weight-chunk group (hc's per DMA)
GD = 4                      # fc2 weight-chunk group (dp's per DMA)

F32 = mybir.dt.float32
F16 = mybir.dt.float16
NPT = np.float16
FPR = F16                   # routed expert weight dtype
ACT = mybir.ActivationFunctionType

LAST_RESULTS = None
_NC_CACHE = {}


def _build_nc(CA, CB, reps=1):
    """SPMD program: routed blocks (capacities CA, CB) + shared block."""
    nc = bacc.Bacc(None, target_bir_lowering=False)

    blocks = []
    for i, cap in enumerate((CA, CB)):
        blocks.append((
            nc.declare_dram_parameter(f"w1e{i}", [NH, P, 2 * KD * P], FPR, isOutput=False),
            nc.declare_dram_parameter(f"w2e{i}", [MD, P, NH * P], FPR, isOutput=False),
            NH,
            nc.declare_dram_parameter(f"xg{i}", [P, KD * cap], F16, isOutput=False),
            cap,
            nc.declare_dram_parameter(f"zg{i}", [P, MD * cap], F16, isOutput=True),
        ))
    blocks.append((
        nc.declare_dram_parameter("ws1", [NHS, P, 2 * KD * P], F16, isOutput=False),
        nc.declare_dram_parameter("ws2", [MD, P, NHS * P], F16, isOutput=False),
        NHS,
        nc.declare_dram_parameter("xd", [P, KD * TDP], F16, isOutput=False),
        TDP,
        nc.declare_dram_parameter("zs", [P, MD * TDP], F16, isOutput=True),
    ))

    with tile.TileContext(nc) as tc:
        with (
            tc.tile_pool(name="xpool", bufs=1) as xpool,
            tc.tile_pool(name="wpool", bufs=1) as wpool,
            tc.tile_pool(name="hpool", bufs=1) as hpool,
            tc.tile_pool(name="spool", bufs=2) as spool,
            tc.tile_pool(name="opool", bufs=1) as opool,
            tc.tile_pool(name="psu", bufs=2, space="PSUM") as psu,
            tc.tile_pool(name="psg", bufs=2, space="PSUM") as psg,
            tc.tile_pool(name="psz", bufs=2, space="PSUM") as psz,
        ):
            pools = (xpool, wpool, hpool, spool, opool, psu, psg, psz)
            loop_cm = tc.For_i(0, reps, 1) if reps > 1 else contextlib.nullcontext()
            with loop_cm:
                _emit_body(nc, blocks, pools)
    nc.finalize()
    return nc


def _emit_body(nc, blocks, pools):
    xpool, wpool, hpool, spool, opool, psu, psg, psz = pools
    for bi, (w1, w2, NHb, xt, C, zt) in enumerate(blocks):
        wdt = w1.dtype if hasattr(w1, "dtype") else FPR
        w1_a = w1.ap()
        w2_a = w2.ap()
        xt_a = xt.ap().rearrange("p (k c) -> p k c", k=KD)
        zt_a = zt.ap().rearrange("p (m c) -> p m c", m=MD)

        x_tile = xpool.tile([P, KD, C], F16, tag=f"x{bi}")
        nc.sync.dma_start(x_tile[:], xt_a)
        h_tile = hpool.tile([P, NHb, C], F16, tag=f"h{bi}")

        # fc1 weight chunks, in consumption order on the sync FIFO
        w1ts = []
        for cc in range(NHb // GH):
            w1t = wpool.tile([P, GH, 2, KD, P], wdt, tag=f"w1_{bi}_{cc}")
            nc.sync.dma_start(
                w1t[:],
                w1_a[GH * cc:GH * (cc + 1)].rearrange(
                    "h p (s k f) -> p h s k f", s=2, k=KD))
            w1ts.append(w1t)

        for hc in range(NHb):
            w1t = w1ts[hc // GH]
            g = hc % GH
            for c0 in range(0, C, 512):
                c1 = min(C, c0 + 512)
                ps_u = psu.tile([P, c1 - c0], F32, tag="psu")
                ps_g = psg.tile([P, c1 - c0], F32, tag="psg")
                for k in range(KD):
                    nc.tensor.matmul(ps_u[:], w1t[:, g, 0, k], x_tile[:, k, c0:c1],
                                     start=(k == 0), stop=(k == KD - 1))
                for k in range(KD):
                    nc.tensor.matmul(ps_g[:], w1t[:, g, 1, k], x_tile[:, k, c0:c1],
                                     start=(k == 0), stop=(k == KD - 1))
                sil = spool.tile([P, c1 - c0], F32, tag="sil")
                nc.scalar.activation(sil[:], ps_g[:], ACT.Silu)
                nc.vector.tensor_mul(h_tile[:, hc, c0:c1], ps_u[:], sil[:])

        o_tile = opool.tile([P, MD, C], F16, tag=f"o{bi}")
        for cc in range(MD // GD):
            w2t = wpool.tile([P, GD, NHb, P], wdt, tag=f"w2_{bi}_{cc}")
            nc.sync.dma_start(
                w2t[:],
                w2_a[GD * cc:GD * (cc + 1)].rearrange(
                    "d p (k f) -> p d k f", k=NHb))
            for d in range(GD):
                for c0 in range(0, C, 512):
                    c1 = min(C, c0 + 512)
                    ps_z = psz.tile([P, c1 - c0], F32, tag="psz")
                    for k in range(NHb):
                        nc.tensor.matmul(ps_z[:], w2t[:, d, k], h_tile[:, k, c0:c1],
                                         start=(k == 0), stop=(k == NHb - 1))
                    nc.vector.tensor_copy(o_tile[:, GD * cc + d, c0:c1], ps_z[:])
        nc.scalar.dma_start(zt_a, o_tile[:])


def _route(xf, Wg):
    """Host gate: softmax over expert logits, top-2 (ties -> lower index,
    matching jax.lax.top_k)."""
    logits = xf @ Wg.T
    m = logits.max(axis=-1, keepdims=True)
    p = np.exp(logits - m)
    scores = p / p.sum(axis=-1, keepdims=True)
    i1 = scores.argmax(axis=-1)
    rows = np.arange(T)
    s1 = scores[rows, i1]
    masked = scores.copy()
    masked[rows, i1] = -np.inf
    i2 = masked.argmax(axis=-1)
    s2 = scores[rows, i2]
    return i1, s1 * ROUTE_SCALE, i2, s2 * ROUTE_SCALE


def _pack_w1(W1b, HB, npt):
    """[D, 2*HB] -> [NHb, P, 2*KD*P]; chunk hc holds [ki, half, ko, f]."""
    NHb = HB // P
    Ar = W1b.reshape(KD, P, 2, NHb, P)             # [ko, ki, half, hc, f]
    return np.ascontiguousarray(
        Ar.transpose(3, 1, 2, 0, 4).reshape(NHb, P, 2 * KD * P).astype(npt))


def _pack_w2(W2b, HB, npt):
    """[HB, D] -> [MD, P, NHb*P]."""
    NHb = HB // P
    Br = W2b.reshape(NHb, P, MD, P)                # [ko, ki, dc, f]
    return np.ascontiguousarray(
        Br.transpose(2, 1, 0, 3).reshape(MD, P, NHb * P).astype(npt))


def _pack_x(xTb, C):
    """[D, n] -> [P, KD*C] (zero-pads the token dim to C)."""
    n = xTb.shape[1]
    out = np.zeros((P, KD, C), dtype=np.float16)
    out[:, :, :n] = xTb.reshape(KD, P, n).transpose(1, 0, 2)
    return out.reshape(P, KD * C)


def _r32(n):
    return max(32, -(-n // 32) * 32)


def prepare(x, Wg, W1, W2, Ws1, Ws2):
    """Host routing + per-core input maps.

    Returns (in_maps, toks, wts, assign, CA, CB) where assign[c] is the
    (expertA, expertB) pair owned by core c."""
    x = np.asarray(x, dtype=np.float32)
    Wg = np.asarray(Wg, dtype=np.float32)
    W1 = np.asarray(W1, dtype=np.float32)
    W2 = np.asarray(W2, dtype=np.float32)
    Ws1 = np.asarray(Ws1, dtype=np.float32)
    Ws2 = np.asarray(Ws2, dtype=np.float32)

    xf = np.ascontiguousarray(x.reshape(T, D))
    i1, s1, i2, s2 = _route(xf, Wg)

    toks, wts = [], []
    for e in range(E):
        sel = np.where((i1 == e) | (i2 == e))[0]
        toks.append(sel)
        wts.append(np.where(i1[sel] == e, s1[sel], s2[sel]).astype(np.float32))

    # Pair the 8 largest with the 8 smallest: slot A holds the big ones.
    order = sorted(range(E), key=lambda e: -len(toks[e]))
    assign = [(order[c], order[E - 1 - c]) for c in range(N_CORES)]
    CA = _r32(max(len(toks[a]) for a, _ in assign))
    CB = _r32(max(len(toks[b]) for _, b in assign))

    ws1p = _pack_w1(Ws1, HS, NPT)
    ws2p = _pack_w2(Ws2, HS, NPT)
    in_maps = []
    for c in range(N_CORES):
        im = {"ws1": ws1p, "ws2": ws2p,
              "xd": _pack_x(np.ascontiguousarray(xf[c * TDP:(c + 1) * TDP].T), TDP)}
        for i, (e, cap) in enumerate(zip(assign[c], (CA, CB))):
            im[f"w1e{i}"] = _pack_w1(W1[e], H, NPT)
            im[f"w2e{i}"] = _pack_w2(W2[e], H, NPT)
            im[f"xg{i}"] = _pack_x(xf[toks[e]].T, cap)
        in_maps.append(im)
    return in_maps, toks, wts, assign, CA, CB


def kernel(x, Wg, W1, W2, Ws1, Ws2):
    global LAST_RESULTS
    in_maps, toks, wts, assign, CA, CB = prepare(x, Wg, W1, W2, Ws1, Ws2)

    key = (CA, CB)
    if key not in _NC_CACHE:
        _NC_CACHE[key] = _build_nc(CA, CB)
    nc = _NC_CACHE[key]

    try:
        LAST_RESULTS = run_bass_kernel_spmd(nc, in_maps, list(range(N_CORES)))
    except Exception:
        # transient NRT device errors have been observed; retry once
        LAST_RESULTS = run_bass_kernel_spmd(nc, in_maps, list(range(N_CORES)))
    res = LAST_RESULTS.results

    out = np.zeros((T, D), dtype=np.float32)
    for c in range(N_CORES):
        for i, (e, cap) in enumerate(zip(assign[c], (CA, CB))):
            n = len(toks[e])
            z = res[c][f"zg{i}"].astype(np.float32).reshape(P, MD, cap)
            z = z.transpose(1, 0, 2).reshape(D, cap)[:, :n]
            out[toks[e]] += wts[e][:, None] * z.T
        zs = res[c]["zs"].astype(np.float32).reshape(P, MD, TDP)
        out[c * TDP:(c + 1) * TDP] += zs.transpose(1, 0, 2).reshape(D, TDP).T
    return out.reshape(B, S, D)


# revision 3
# speedup vs baseline: 1.1142x; 1.0418x over previous
"""MoE (16 routed experts, top-2, + shared expert) on 8 Trainium2 cores.

Strategy (expert-parallel, host-side dispatch):
  - Host computes the gate (softmax + top-2) and gathers each expert's
    tokens; experts are permuted so each core owns one "big" and one
    "small" expert (slot capacities CA >= CB), minimizing pad columns.
  - The shared expert is split across core pairs: core c computes the
    hidden slice [q*HS/2, (q+1)*HS/2) (q = c%2) of the shared expert
    for the 512 tokens owned by its pair; the host adds the two
    halves. This halves the (otherwise 8x-replicated) shared-expert
    weight traffic per core.
  - Each core runs the same program: three gated-MLP blocks
    (expertA, expertB, shared-half) in a transposed layout
        zT = W2^T @ (u * silu(g)),  [u;g]^T = W1^T @ xT
    so no on-chip transposes are needed anywhere.
  - Routed expert weights travel as fp8e4 (e4m3) scaled by 128; the
    silu input is descaled on the ACT engine (activation scale=1/128)
    and the host divides the routed outputs by 128^2. The shared
    expert stays f16. End-to-end rel err ~1.2e-2 (gate is 2e-2).
  - Weights are packed flat per partition ([P, n]) so every DMA reads
    one contiguous run per partition; weight loads stream on the sync
    (SP) HWDGE FIFO in exactly the order compute consumes them, while
    x loads and z stores ride the scalar (ACT) FIFO.
"""

import sys

for _p in ("/opt/trn_rl_repo", "/root/.axon_site/_ro/trn_rl_repo"):
    if _p not in sys.path:
        sys.path.insert(0, _p)

import contextlib
import os

import numpy as np
import ml_dtypes

import concourse.bass as bass  # noqa: F401
import concourse.tile as tile
from concourse import bacc, mybir
from concourse.bass_utils import run_bass_kernel_spmd

try:  # tracing needs the axon NTFF hook; absent in some containers
    from antenv import axon_hooks as _axon_hooks  # noqa: F401
except ImportError:
    os.environ.setdefault("BASS_NEVER_TRACE", "1")

B, S, D = 2, 1024, 1024
H = 512           # routed expert hidden
HS = 1024         # shared expert hidden
HL = HS // 2      # shared hidden per core (pair-split)
E = 16
ROUTE_SCALE = 1.0
T = B * S
N_CORES = 8
TDP2 = 2 * (T // N_CORES)   # shared-expert tokens per core pair (512)
P = 128
KD = D // P                 # fc1 contraction chunks
MD = D // P                 # fc2 output chunks
NH = H // P                 # routed fc1 output chunks per half (4)
NHL = HL // P               # shared-half fc1 output chunks per half (4)
GH = 2                      # fc1 weight-chunk group (hc's per DMA)
GD = 4                      # fc2 weight-chunk group (dp's per DMA)

F32 = mybir.dt.float32
F16 = mybir.dt.float16
FP8 = mybir.dt.float8e4
NP16 = np.float16
NP8 = ml_dtypes.float8_e4m3fn
W8SCALE = 128.0             # fp8 weight pre-scale (per routed matmul)
ROUTED_FP8 = True
ACT = mybir.ActivationFunctionType

LAST_RESULTS = None
_NC_CACHE = {}


def _build_nc(CA, CB, reps=1):
    """SPMD program: routed blocks (capacities CA, CB) + shared-half block."""
    nc = bacc.Bacc(None, target_bir_lowering=False)
    wdt_r = FP8 if ROUTED_FP8 else F16
    asc_r = (1.0 / W8SCALE) if ROUTED_FP8 else 1.0

    blocks = []
    for i, cap in enumerate((CA, CB)):
        blocks.append((
            nc.declare_dram_parameter(f"w1e{i}", [P, NH * 2 * KD * P], wdt_r, isOutput=False),
            nc.declare_dram_parameter(f"w2e{i}", [P, MD * NH * P], wdt_r, isOutput=False),
            NH,
            nc.declare_dram_parameter(f"xg{i}", [P, KD * cap], F16, isOutput=False),
            cap,
            nc.declare_dram_parameter(f"zg{i}", [P, MD * cap], F16, isOutput=True),
            wdt_r, asc_r,
        ))
    blocks.append((
        nc.declare_dram_parameter("ws1", [P, NHL * 2 * KD * P], F16, isOutput=False),
        nc.declare_dram_parameter("ws2", [P, MD * NHL * P], F16, isOutput=False),
        NHL,
        nc.declare_dram_parameter("xd", [P, KD * TDP2], F16, isOutput=False),
        TDP2,
        nc.declare_dram_parameter("zs", [P, MD * TDP2], F16, isOutput=True),
        F16, 1.0,
    ))

    with tile.TileContext(nc) as tc:
        with (
            tc.tile_pool(name="xpool", bufs=1) as xpool,
            tc.tile_pool(name="wpool", bufs=1) as wpool,
            tc.tile_pool(name="hpool", bufs=1) as hpool,
            tc.tile_pool(name="spool", bufs=2) as spool,
            tc.tile_pool(name="opool", bufs=1) as opool,
            tc.tile_pool(name="psu", bufs=2, space="PSUM") as psu,
            tc.tile_pool(name="psg", bufs=2, space="PSUM") as psg,
            tc.tile_pool(name="psz", bufs=2, space="PSUM") as psz,
        ):
            pools = (xpool, wpool, hpool, spool, opool, psu, psg, psz)
            loop_cm = tc.For_i(0, reps, 1) if reps > 1 else contextlib.nullcontext()
            with loop_cm:
                _emit_body(nc, blocks, pools)
    nc.finalize()
    return nc


def _emit_body(nc, blocks, pools):
    xpool, wpool, hpool, spool, opool, psu, psg, psz = pools
    for bi, (w1, w2, NHb, xt, C, zt, wdt, ascale) in enumerate(blocks):
        w1_a = w1.ap()
        w2_a = w2.ap()
        xt_a = xt.ap().rearrange("p (k c) -> p k c", k=KD)
        zt_a = zt.ap().rearrange("p (m c) -> p m c", m=MD)

        x_tile = xpool.tile([P, KD, C], F16, tag=f"x{bi}")
        nc.scalar.dma_start(x_tile[:], xt_a)
        h_tile = hpool.tile([P, NHb, C], F16, tag=f"h{bi}")

        # fc1 weight chunks, in consumption order on the sync FIFO
        csz1 = GH * 2 * KD * P
        w1ts = []
        for cc in range(NHb // GH):
            w1t = wpool.tile([P, GH, 2, KD, P], wdt, tag=f"w1_{bi}_{cc}")
            nc.sync.dma_start(
                w1t[:],
                w1_a[:, cc * csz1:(cc + 1) * csz1].rearrange(
                    "p (h s k f) -> p h s k f", h=GH, s=2, k=KD))
            w1ts.append(w1t)

        for hc in range(NHb):
            w1t = w1ts[hc // GH]
            g = hc % GH
            for c0 in range(0, C, 512):
                c1 = min(C, c0 + 512)
                ps_u = psu.tile([P, c1 - c0], F32, tag="psu")
                ps_g = psg.tile([P, c1 - c0], F32, tag="psg")
                for k in range(KD):
                    nc.tensor.matmul(ps_u[:], w1t[:, g, 0, k], x_tile[:, k, c0:c1],
                                     start=(k == 0), stop=(k == KD - 1))
                for k in range(KD):
                    nc.tensor.matmul(ps_g[:], w1t[:, g, 1, k], x_tile[:, k, c0:c1],
                                     start=(k == 0), stop=(k == KD - 1))
                sil = spool.tile([P, c1 - c0], F32, tag="sil")
                if ascale != 1.0:
                    nc.scalar.activation(sil[:], ps_g[:], ACT.Silu, scale=ascale)
                else:
                    nc.scalar.activation(sil[:], ps_g[:], ACT.Silu)
                nc.vector.tensor_mul(h_tile[:, hc, c0:c1], ps_u[:], sil[:])

        o_tile = opool.tile([P, MD, C], F16, tag=f"o{bi}")
        csz2 = GD * NHb * P
        for cc in range(MD // GD):
            w2t = wpool.tile([P, GD, NHb, P], wdt, tag=f"w2_{bi}_{cc}")
            nc.sync.dma_start(
                w2t[:],
                w2_a[:, cc * csz2:(cc + 1) * csz2].rearrange(
                    "p (d k f) -> p d k f", d=GD, k=NHb))
            for d in range(GD):
                for c0 in range(0, C, 512):
                    c1 = min(C, c0 + 512)
                    ps_z = psz.tile([P, c1 - c0], F32, tag="psz")
                    for k in range(NHb):
                        nc.tensor.matmul(ps_z[:], w2t[:, d, k], h_tile[:, k, c0:c1],
                                         start=(k == 0), stop=(k == NHb - 1))
                    nc.vector.tensor_copy(o_tile[:, GD * cc + d, c0:c1], ps_z[:])
        nc.scalar.dma_start(zt_a, o_tile[:])


def _route(xf, Wg):
    """Host gate: softmax over expert logits, top-2 (ties -> lower index,
    matching jax.lax.top_k)."""
    logits = xf @ Wg.T
    m = logits.max(axis=-1, keepdims=True)
    p = np.exp(logits - m)
    scores = p / p.sum(axis=-1, keepdims=True)
    i1 = scores.argmax(axis=-1)
    rows = np.arange(T)
    s1 = scores[rows, i1]
    masked = scores.copy()
    masked[rows, i1] = -np.inf
    i2 = masked.argmax(axis=-1)
    s2 = scores[rows, i2]
    return i1, s1 * ROUTE_SCALE, i2, s2 * ROUTE_SCALE


def _pack_w1(W1b, HB, npt, scale=1.0):
    """[D, 2*HB] -> [P, NHb*2*KD*P] flat per partition, (hc, half, ko, f)."""
    NHb = HB // P
    Ar = (W1b * scale).reshape(KD, P, 2, NHb, P)   # [ko, ki, half, hc, f]
    return np.ascontiguousarray(
        Ar.transpose(1, 3, 2, 0, 4).reshape(P, NHb * 2 * KD * P).astype(npt))


def _pack_w2(W2b, HB, npt, scale=1.0):
    """[HB, D] -> [P, MD*NHb*P] flat per partition, (dc, ko, f)."""
    NHb = HB // P
    Br = (W2b * scale).reshape(NHb, P, MD, P)      # [ko, ki, dc, f]
    return np.ascontiguousarray(
        Br.transpose(1, 2, 0, 3).reshape(P, MD * NHb * P).astype(npt))


def _pack_x(xTb, C):
    """[D, n] -> [P, KD*C] (zero-pads the token dim to C)."""
    n = xTb.shape[1]
    out = np.zeros((P, KD, C), dtype=np.float16)
    out[:, :, :n] = xTb.reshape(KD, P, n).transpose(1, 0, 2)
    return out.reshape(P, KD * C)


def _r32(n):
    return max(32, -(-n // 32) * 32)


def prepare(x, Wg, W1, W2, Ws1, Ws2):
    """Host routing + per-core input maps.

    Returns (in_maps, toks, wts, assign, CA, CB) where assign[c] is the
    (expertA, expertB) pair owned by core c."""
    x = np.asarray(x, dtype=np.float32)
    Wg = np.asarray(Wg, dtype=np.float32)
    W1 = np.asarray(W1, dtype=np.float32)
    W2 = np.asarray(W2, dtype=np.float32)
    Ws1 = np.asarray(Ws1, dtype=np.float32)
    Ws2 = np.asarray(Ws2, dtype=np.float32)

    xf = np.ascontiguousarray(x.reshape(T, D))
    i1, s1, i2, s2 = _route(xf, Wg)

    toks, wts = [], []
    for e in range(E):
        sel = np.where((i1 == e) | (i2 == e))[0]
        toks.append(sel)
        wts.append(np.where(i1[sel] == e, s1[sel], s2[sel]).astype(np.float32))

    # Pair the 8 largest with the 8 smallest: slot A holds the big ones.
    order = sorted(range(E), key=lambda e: -len(toks[e]))
    assign = [(order[c], order[E - 1 - c]) for c in range(N_CORES)]
    CA = _r32(max(len(toks[a]) for a, _ in assign))
    CB = _r32(max(len(toks[b]) for _, b in assign))

    if ROUTED_FP8:
        npt_r, wscale = NP8, W8SCALE
    else:
        npt_r, wscale = NP16, 1.0

    # Shared expert, split by hidden half: q=0 -> cols [0,HL), q=1 -> [HL,HS)
    ws1p, ws2p = [], []
    for q in range(2):
        W1s = np.concatenate(
            [Ws1[:, q * HL:(q + 1) * HL], Ws1[:, HS + q * HL:HS + (q + 1) * HL]],
            axis=1)
        ws1p.append(_pack_w1(W1s, HL, NP16))
        ws2p.append(_pack_w2(Ws2[q * HL:(q + 1) * HL], HL, NP16))

    in_maps = []
    for c in range(N_CORES):
        grp, q = c // 2, c % 2
        xd = _pack_x(np.ascontiguousarray(
            xf[grp * TDP2:(grp + 1) * TDP2].T), TDP2)
        im = {"ws1": ws1p[q], "ws2": ws2p[q], "xd": xd}
        for i, (e, cap) in enumerate(zip(assign[c], (CA, CB))):
            im[f"w1e{i}"] = _pack_w1(W1[e], H, npt_r, wscale)
            im[f"w2e{i}"] = _pack_w2(W2[e], H, npt_r, wscale)
            im[f"xg{i}"] = _pack_x(xf[toks[e]].T, cap)
        in_maps.append(im)
    return in_maps, toks, wts, assign, CA, CB


def kernel(x, Wg, W1, W2, Ws1, Ws2):
    global LAST_RESULTS
    in_maps, toks, wts, assign, CA, CB = prepare(x, Wg, W1, W2, Ws1, Ws2)

    key = (CA, CB)
    if key not in _NC_CACHE:
        _NC_CACHE[key] = _build_nc(CA, CB)
    nc = _NC_CACHE[key]

    try:
        LAST_RESULTS = run_bass_kernel_spmd(nc, in_maps, list(range(N_CORES)))
    except Exception:
        # transient NRT device errors have been observed; retry once
        LAST_RESULTS = run_bass_kernel_spmd(nc, in_maps, list(range(N_CORES)))
    res = LAST_RESULTS.results

    zscale = 1.0 / (W8SCALE * W8SCALE) if ROUTED_FP8 else 1.0
    out = np.zeros((T, D), dtype=np.float32)
    for c in range(N_CORES):
        grp = c // 2
        for i, (e, cap) in enumerate(zip(assign[c], (CA, CB))):
            n = len(toks[e])
            z = res[c][f"zg{i}"].astype(np.float32).reshape(P, MD, cap)
            z = z.transpose(1, 0, 2).reshape(D, cap)[:, :n]
            out[toks[e]] += (zscale * wts[e])[:, None] * z.T
        zs = res[c]["zs"].astype(np.float32).reshape(P, MD, TDP2)
        out[grp * TDP2:(grp + 1) * TDP2] += zs.transpose(1, 0, 2).reshape(D, TDP2).T
    return out.reshape(B, S, D)


# revision 12
# speedup vs baseline: 1.2245x; 1.0990x over previous
"""MoE (16 routed experts, top-2, + shared expert) on 8 Trainium2 cores.

Strategy (expert-parallel, host-side dispatch):
  - Host computes the gate (softmax + top-2) and gathers each expert's
    tokens; experts are permuted so each core owns one "big" and one
    "small" expert (slot capacities CA >= CB), minimizing pad columns.
  - The shared expert is split across core pairs: core c computes the
    hidden slice [q*HS/2, (q+1)*HS/2) (q = c%2) of the shared expert
    for the 512 tokens owned by its pair; the host adds the two
    halves. This halves the (otherwise 8x-replicated) shared-expert
    weight traffic per core.
  - Each core runs the same program: three gated-MLP blocks
    (expertA, expertB, shared-half) in a transposed layout
        zT = W2^T @ (u * silu(g)),  [u;g]^T = W1^T @ xT
    so no on-chip transposes are needed anywhere.
  - Routed expert weights travel as fp8e4 (e4m3) scaled by 128; the
    silu input is descaled on the ACT engine (activation scale=1/128)
    and the host divides the routed outputs by 128^2. The shared
    expert stays f16. End-to-end rel err ~1.2e-2 (gate is 2e-2).
  - Weights are packed flat per partition ([P, n]) so every DMA reads
    one contiguous run per partition; weight loads stream on the sync
    (SP) HWDGE FIFO in exactly the order compute consumes them, while
    x loads and z stores ride the scalar (ACT) FIFO.
"""

import sys

for _p in ("/opt/trn_rl_repo", "/root/.axon_site/_ro/trn_rl_repo"):
    if _p not in sys.path:
        sys.path.insert(0, _p)

import contextlib
import os

import numpy as np
import ml_dtypes

import concourse.bass as bass  # noqa: F401
import concourse.tile as tile
from concourse import bacc, mybir
from concourse.bass_utils import run_bass_kernel_spmd

try:  # tracing needs the axon NTFF hook; absent in some containers
    from antenv import axon_hooks as _axon_hooks  # noqa: F401
except ImportError:
    os.environ.setdefault("BASS_NEVER_TRACE", "1")

B, S, D = 2, 1024, 1024
H = 512           # routed expert hidden
HS = 1024         # shared expert hidden
HL = HS // 2      # shared hidden per core (pair-split)
E = 16
ROUTE_SCALE = 1.0
T = B * S
N_CORES = 8
TDP2 = 2 * (T // N_CORES)   # shared-expert tokens per core pair (512)
P = 128
KD = D // P                 # fc1 contraction chunks
MD = D // P                 # fc2 output chunks
NH = H // P                 # routed fc1 output chunks per half (4)
NHL = HL // P               # shared-half fc1 output chunks per half (4)
GH = 2                      # fc1 weight-chunk group (hc's per DMA)
GD = 4                      # fc2 weight-chunk group (dp's per DMA)

F32 = mybir.dt.float32
F16 = mybir.dt.float16
FP8 = mybir.dt.float8e4
NP16 = np.float16
NP8 = ml_dtypes.float8_e4m3fn
W8SCALE = 128.0             # fp8 weight pre-scale (per routed matmul)
ROUTED_FP8 = True
DR_FC1 = True               # DoubleRow (fp8 x + fp8 w1) on routed fc1
ACT = mybir.ActivationFunctionType
DROW = mybir.MatmulPerfMode.DoubleRow

LAST_RESULTS = None
_NC_CACHE = {}


def _build_nc(CA, CB, reps=1):
    """SPMD program: routed blocks (capacities CA, CB) + shared-half block."""
    nc = bacc.Bacc(None, target_bir_lowering=False)
    wdt_r = FP8 if ROUTED_FP8 else F16
    asc_r = (1.0 / W8SCALE) if ROUTED_FP8 else 1.0

    xdt_r = FP8 if (ROUTED_FP8 and DR_FC1) else F16
    dr_r = ROUTED_FP8 and DR_FC1
    rblocks = []
    for i, cap in enumerate((CA, CB)):
        rblocks.append((
            nc.declare_dram_parameter(f"w1e{i}", [P, NH * 2 * KD * P], wdt_r, isOutput=False),
            nc.declare_dram_parameter(f"w2e{i}", [P, MD * NH * P], wdt_r, isOutput=False),
            NH,
            nc.declare_dram_parameter(f"xg{i}", [P, KD * cap], xdt_r, isOutput=False),
            cap,
            nc.declare_dram_parameter(f"zg{i}", [P, MD * cap], F16, isOutput=True),
            wdt_r, asc_r, xdt_r, dr_r,
        ))
    sblock = (
        nc.declare_dram_parameter("ws1", [P, NHL * 2 * KD * P], F16, isOutput=False),
        nc.declare_dram_parameter("ws2", [P, MD * NHL * P], F16, isOutput=False),
        NHL,
        nc.declare_dram_parameter("xd", [P, KD * TDP2], F16, isOutput=False),
        TDP2,
        nc.declare_dram_parameter("zs", [P, MD * TDP2], F16, isOutput=True),
        F16, 1.0, F16, False,
    )
    # order: big expert first (small head), small expert last (small tail)
    blocks = [rblocks[0], sblock, rblocks[1]]

    with tile.TileContext(nc) as tc:
        with (
            tc.tile_pool(name="xpool", bufs=1) as xpool,
            tc.tile_pool(name="wpool", bufs=1) as wpool,
            tc.tile_pool(name="hpool", bufs=1) as hpool,
            tc.tile_pool(name="spool", bufs=2) as spool,
            tc.tile_pool(name="opool", bufs=1) as opool,
            tc.tile_pool(name="psu", bufs=3, space="PSUM") as psu,
            tc.tile_pool(name="psg", bufs=3, space="PSUM") as psg,
            tc.tile_pool(name="psz", bufs=2, space="PSUM") as psz,
        ):
            pools = (xpool, wpool, hpool, spool, opool, psu, psg, psz)
            loop_cm = tc.For_i(0, reps, 1) if reps > 1 else contextlib.nullcontext()
            with loop_cm:
                _emit_body(nc, blocks, pools)
    nc.finalize()
    return nc


def _emit_body(nc, blocks, pools):
    xpool, wpool, hpool, spool, opool, psu, psg, psz = pools
    last = len(blocks) - 1
    for bi, (w1, w2, NHb, xt, C, zt, wdt, ascale, xdt, dr) in enumerate(blocks):
        w1_a = w1.ap()
        w2_a = w2.ap()
        xt_a = xt.ap().rearrange("p (k c) -> p k c", k=KD)
        zt_a = zt.ap().rearrange("p (m c) -> p m c", m=MD)

        x_tile = xpool.tile([P, KD, C], xdt, tag=f"x{bi}")
        nc.scalar.dma_start(x_tile[:], xt_a)
        h_tile = hpool.tile([P, NHb, C], F16, tag=f"h{bi}")

        # fc1 weight chunks, in consumption order on the sync FIFO
        csz1 = GH * 2 * KD * P
        w1ts = []
        for cc in range(NHb // GH):
            if dr:
                w1t = wpool.tile([P, GH, 2, KD // 2, 2, P], wdt, tag=f"w1_{bi}_{cc}")
                w1t_src = w1_a[:, cc * csz1:(cc + 1) * csz1].rearrange(
                    "p (h s j t f) -> p h s j t f", h=GH, s=2, j=KD // 2, t=2)
            else:
                w1t = wpool.tile([P, GH, 2, KD, P], wdt, tag=f"w1_{bi}_{cc}")
                w1t_src = w1_a[:, cc * csz1:(cc + 1) * csz1].rearrange(
                    "p (h s k f) -> p h s k f", h=GH, s=2, k=KD)
            nc.sync.dma_start(w1t[:], w1t_src)
            w1ts.append(w1t)

        for hc in range(NHb):
            w1t = w1ts[hc // GH]
            g = hc % GH
            for c0 in range(0, C, 512):
                c1 = min(C, c0 + 512)
                ps_u = psu.tile([P, c1 - c0], F32, tag="psu")
                ps_g = psg.tile([P, c1 - c0], F32, tag="psg")
                if dr:
                    for j in range(KD // 2):
                        nc.tensor.matmul(ps_u[:], w1t[:, g, 0, j],
                                         x_tile[:, 2 * j:2 * j + 2, c0:c1],
                                         start=(j == 0), stop=(j == KD // 2 - 1),
                                         perf_mode=DROW)
                    for j in range(KD // 2):
                        nc.tensor.matmul(ps_g[:], w1t[:, g, 1, j],
                                         x_tile[:, 2 * j:2 * j + 2, c0:c1],
                                         start=(j == 0), stop=(j == KD // 2 - 1),
                                         perf_mode=DROW)
                else:
                    for k in range(KD):
                        nc.tensor.matmul(ps_u[:], w1t[:, g, 0, k], x_tile[:, k, c0:c1],
                                         start=(k == 0), stop=(k == KD - 1))
                    for k in range(KD):
                        nc.tensor.matmul(ps_g[:], w1t[:, g, 1, k], x_tile[:, k, c0:c1],
                                         start=(k == 0), stop=(k == KD - 1))
                sil = spool.tile([P, c1 - c0], F32, tag="sil")
                if ascale != 1.0:
                    nc.scalar.activation(sil[:], ps_g[:], ACT.Silu, scale=ascale)
                else:
                    nc.scalar.activation(sil[:], ps_g[:], ACT.Silu)
                nc.vector.tensor_mul(h_tile[:, hc, c0:c1], ps_u[:], sil[:])

        o_tile = opool.tile([P, MD, C], F16, tag=f"o{bi}")
        csz2 = GD * NHb * P
        for cc in range(MD // GD):
            w2t = wpool.tile([P, GD, NHb, P], wdt, tag=f"w2_{bi}_{cc}")
            nc.sync.dma_start(
                w2t[:],
                w2_a[:, cc * csz2:(cc + 1) * csz2].rearrange(
                    "p (d k f) -> p d k f", d=GD, k=NHb))
            for d in range(GD):
                for c0 in range(0, C, 512):
                    c1 = min(C, c0 + 512)
                    ps_z = psz.tile([P, c1 - c0], F32, tag="psz")
                    for k in range(NHb):
                        nc.tensor.matmul(ps_z[:], w2t[:, d, k], h_tile[:, k, c0:c1],
                                         start=(k == 0), stop=(k == NHb - 1))
                    nc.vector.tensor_copy(o_tile[:, GD * cc + d, c0:c1], ps_z[:])
            if bi == last:
                # split the final store so most of it overlaps compute
                nc.scalar.dma_start(zt_a[:, GD * cc:GD * (cc + 1)],
                                    o_tile[:, GD * cc:GD * (cc + 1)])
        if bi != last:
            nc.scalar.dma_start(zt_a, o_tile[:])


def _route(xf, Wg):
    """Host gate: softmax over expert logits, top-2 (ties -> lower index,
    matching jax.lax.top_k)."""
    logits = xf @ Wg.T
    m = logits.max(axis=-1, keepdims=True)
    p = np.exp(logits - m)
    scores = p / p.sum(axis=-1, keepdims=True)
    i1 = scores.argmax(axis=-1)
    rows = np.arange(T)
    s1 = scores[rows, i1]
    masked = scores.copy()
    masked[rows, i1] = -np.inf
    i2 = masked.argmax(axis=-1)
    s2 = scores[rows, i2]
    return i1, s1 * ROUTE_SCALE, i2, s2 * ROUTE_SCALE


def _pack_w1(W1b, HB, npt, scale=1.0):
    """[D, 2*HB] -> [P, NHb*2*KD*P] flat per partition, (hc, half, ko, f).
    (For DoubleRow the ko axis is viewed as (ko//2, ko%2) — same layout.)"""
    NHb = HB // P
    Ar = (W1b * scale).reshape(KD, P, 2, NHb, P)   # [ko, ki, half, hc, f]
    return np.ascontiguousarray(
        Ar.transpose(1, 3, 2, 0, 4).reshape(P, NHb * 2 * KD * P).astype(npt))


def _pack_w2(W2b, HB, npt, scale=1.0):
    """[HB, D] -> [P, MD*NHb*P] flat per partition, (dc, ko, f)."""
    NHb = HB // P
    Br = (W2b * scale).reshape(NHb, P, MD, P)      # [ko, ki, dc, f]
    return np.ascontiguousarray(
        Br.transpose(1, 2, 0, 3).reshape(P, MD * NHb * P).astype(npt))


def _pack_x(xTb, C, npt=np.float16):
    """[D, n] -> [P, KD*C] (zero-pads the token dim to C)."""
    n = xTb.shape[1]
    out = np.zeros((P, KD, C), dtype=npt)
    out[:, :, :n] = xTb.reshape(KD, P, n).transpose(1, 0, 2).astype(npt)
    return out.reshape(P, KD * C)


def _r32(n):
    return max(32, -(-n // 32) * 32)


def prepare(x, Wg, W1, W2, Ws1, Ws2):
    """Host routing + per-core input maps.

    Returns (in_maps, toks, wts, assign, CA, CB) where assign[c] is the
    (expertA, expertB) pair owned by core c."""
    x = np.asarray(x, dtype=np.float32)
    Wg = np.asarray(Wg, dtype=np.float32)
    W1 = np.asarray(W1, dtype=np.float32)
    W2 = np.asarray(W2, dtype=np.float32)
    Ws1 = np.asarray(Ws1, dtype=np.float32)
    Ws2 = np.asarray(Ws2, dtype=np.float32)

    xf = np.ascontiguousarray(x.reshape(T, D))
    i1, s1, i2, s2 = _route(xf, Wg)

    toks, wts = [], []
    for e in range(E):
        sel = np.where((i1 == e) | (i2 == e))[0]
        toks.append(sel)
        wts.append(np.where(i1[sel] == e, s1[sel], s2[sel]).astype(np.float32))

    # Pair the 8 largest with the 8 smallest: slot A holds the big ones.
    order = sorted(range(E), key=lambda e: -len(toks[e]))
    assign = [(order[c], order[E - 1 - c]) for c in range(N_CORES)]
    CA = _r32(max(len(toks[a]) for a, _ in assign))
    CB = _r32(max(len(toks[b]) for _, b in assign))

    if ROUTED_FP8:
        npt_r, wscale = NP8, W8SCALE
    else:
        npt_r, wscale = NP16, 1.0
    npt_x = NP8 if (ROUTED_FP8 and DR_FC1) else NP16

    # Shared expert, split by hidden half: q=0 -> cols [0,HL), q=1 -> [HL,HS)
    ws1p, ws2p = [], []
    for q in range(2):
        W1s = np.concatenate(
            [Ws1[:, q * HL:(q + 1) * HL], Ws1[:, HS + q * HL:HS + (q + 1) * HL]],
            axis=1)
        ws1p.append(_pack_w1(W1s, HL, NP16))
        ws2p.append(_pack_w2(Ws2[q * HL:(q + 1) * HL], HL, NP16))

    in_maps = []
    for c in range(N_CORES):
        grp, q = c // 2, c % 2
        xd = _pack_x(np.ascontiguousarray(
            xf[grp * TDP2:(grp + 1) * TDP2].T), TDP2)
        im = {"ws1": ws1p[q], "ws2": ws2p[q], "xd": xd}
        for i, (e, cap) in enumerate(zip(assign[c], (CA, CB))):
            im[f"w1e{i}"] = _pack_w1(W1[e], H, npt_r, wscale)
            im[f"w2e{i}"] = _pack_w2(W2[e], H, npt_r, wscale)
            im[f"xg{i}"] = _pack_x(xf[toks[e]].T, cap, npt_x)
        in_maps.append(im)
    return in_maps, toks, wts, assign, CA, CB


def kernel(x, Wg, W1, W2, Ws1, Ws2):
    global LAST_RESULTS
    in_maps, toks, wts, assign, CA, CB = prepare(x, Wg, W1, W2, Ws1, Ws2)

    key = (CA, CB)
    if key not in _NC_CACHE:
        _NC_CACHE[key] = _build_nc(CA, CB)
    nc = _NC_CACHE[key]

    try:
        LAST_RESULTS = run_bass_kernel_spmd(nc, in_maps, list(range(N_CORES)))
    except Exception:
        # transient NRT device errors have been observed; retry once
        LAST_RESULTS = run_bass_kernel_spmd(nc, in_maps, list(range(N_CORES)))
    res = LAST_RESULTS.results

    zscale = 1.0 / (W8SCALE * W8SCALE) if ROUTED_FP8 else 1.0
    out = np.zeros((T, D), dtype=np.float32)
    for c in range(N_CORES):
        grp = c // 2
        for i, (e, cap) in enumerate(zip(assign[c], (CA, CB))):
            n = len(toks[e])
            z = res[c][f"zg{i}"].astype(np.float32).reshape(P, MD, cap)
            z = z.transpose(1, 0, 2).reshape(D, cap)[:, :n]
            out[toks[e]] += (zscale * wts[e])[:, None] * z.T
        zs = res[c]["zs"].astype(np.float32).reshape(P, MD, TDP2)
        out[grp * TDP2:(grp + 1) * TDP2] += zs.transpose(1, 0, 2).reshape(D, TDP2).T
    return out.reshape(B, S, D)


# revision 13
# speedup vs baseline: 1.3234x; 1.0808x over previous
"""MoE (16 routed experts, top-2, + shared expert) on 8 Trainium2 cores.

Strategy (expert-parallel, host-side dispatch):
  - Host computes the gate (softmax + top-2) and gathers each expert's
    tokens; experts are permuted so each core owns one "big" and one
    "small" expert (slot capacities CA >= CB), minimizing pad columns.
  - The shared expert is split across core pairs: core c computes the
    hidden slice [q*HS/2, (q+1)*HS/2) (q = c%2) of the shared expert
    for the 512 tokens owned by its pair; the host adds the two
    halves. This halves the (otherwise 8x-replicated) shared-expert
    weight traffic per core.
  - Each core runs the same program: three gated-MLP blocks
    (expertA, expertB, shared-half) in a transposed layout
        zT = W2^T @ (u * silu(g)),  [u;g]^T = W1^T @ xT
    so no on-chip transposes are needed anywhere.
  - Routed expert weights travel as fp8e4 (e4m3) scaled by 128; the
    silu input is descaled on the ACT engine (activation scale=1/128)
    and the host divides the routed outputs by 128^2. The shared
    expert stays f16. End-to-end rel err ~1.2e-2 (gate is 2e-2).
  - Weights are packed flat per partition ([P, n]) so every DMA reads
    one contiguous run per partition; weight loads stream on the sync
    (SP) HWDGE FIFO in exactly the order compute consumes them, while
    x loads and z stores ride the scalar (ACT) FIFO.
"""

import sys

for _p in ("/opt/trn_rl_repo", "/root/.axon_site/_ro/trn_rl_repo"):
    if _p not in sys.path:
        sys.path.insert(0, _p)

import contextlib
import os

import numpy as np
import ml_dtypes

import concourse.bass as bass  # noqa: F401
import concourse.tile as tile
from concourse import bacc, mybir
from concourse.bass_utils import run_bass_kernel_spmd

try:  # tracing needs the axon NTFF hook; absent in some containers
    from antenv import axon_hooks as _axon_hooks  # noqa: F401
except ImportError:
    os.environ.setdefault("BASS_NEVER_TRACE", "1")

B, S, D = 2, 1024, 1024
H = 512           # routed expert hidden
HS = 1024         # shared expert hidden
HL = HS // 2      # shared hidden per core (pair-split)
E = 16
ROUTE_SCALE = 1.0
T = B * S
N_CORES = 8
TDP2 = 2 * (T // N_CORES)   # shared-expert tokens per core pair (512)
P = 128
KD = D // P                 # fc1 contraction chunks
MD = D // P                 # fc2 output chunks
NH = H // P                 # routed fc1 output chunks per half (4)
NHL = HL // P               # shared-half fc1 output chunks per half (4)
GH = 2                      # fc1 weight-chunk group (hc's per DMA)
GD = 4                      # fc2 weight-chunk group (dp's per DMA)

F32 = mybir.dt.float32
F16 = mybir.dt.float16
FP8 = mybir.dt.float8e4
NP16 = np.float16
NP8 = ml_dtypes.float8_e4m3fn
W8SCALE = 128.0             # fp8 weight pre-scale (per routed matmul)
ROUTED_FP8 = True
DR_FC1 = True               # DoubleRow (fp8 x + fp8 w1) on routed fc1
ACT = mybir.ActivationFunctionType
DROW = mybir.MatmulPerfMode.DoubleRow

LAST_RESULTS = None
_NC_CACHE = {}


def _build_nc(CA, CB, reps=1):
    """SPMD program: routed blocks (capacities CA, CB) + shared-half block."""
    nc = bacc.Bacc(None, target_bir_lowering=False)
    wdt_r = FP8 if ROUTED_FP8 else F16
    asc_r = (1.0 / W8SCALE) if ROUTED_FP8 else 1.0

    xdt_r = FP8 if (ROUTED_FP8 and DR_FC1) else F16
    dr_r = ROUTED_FP8 and DR_FC1
    rblocks = []
    for i, cap in enumerate((CA, CB)):
        rblocks.append((
            nc.declare_dram_parameter(f"w1e{i}", [P, NH * 2 * KD * P], wdt_r, isOutput=False),
            nc.declare_dram_parameter(f"w2e{i}", [P, MD * NH * P], wdt_r, isOutput=False),
            NH,
            nc.declare_dram_parameter(f"xg{i}", [P, KD * cap], xdt_r, isOutput=False),
            cap,
            nc.declare_dram_parameter(f"zg{i}", [P, MD * cap], F16, isOutput=True),
            wdt_r, asc_r, xdt_r, dr_r,
        ))
    sblock = (
        nc.declare_dram_parameter("ws1", [P, NHL * 2 * KD * P], F16, isOutput=False),
        nc.declare_dram_parameter("ws2", [P, MD * NHL * P], F16, isOutput=False),
        NHL,
        nc.declare_dram_parameter("xd", [P, KD * TDP2], F16, isOutput=False),
        TDP2,
        nc.declare_dram_parameter("zs", [P, MD * TDP2], F16, isOutput=True),
        F16, 1.0, F16, False,
    )
    # order: big expert first (small head), small expert last (small tail)
    blocks = [rblocks[0], sblock, rblocks[1]]

    with tile.TileContext(nc) as tc:
        with (
            tc.tile_pool(name="xpool", bufs=1) as xpool,
            tc.tile_pool(name="wpool", bufs=1) as wpool,
            tc.tile_pool(name="hpool", bufs=1) as hpool,
            tc.tile_pool(name="spool", bufs=2) as spool,
            tc.tile_pool(name="opool", bufs=1) as opool,
            tc.tile_pool(name="psu", bufs=3, space="PSUM") as psu,
            tc.tile_pool(name="psg", bufs=3, space="PSUM") as psg,
            tc.tile_pool(name="psz", bufs=2, space="PSUM") as psz,
        ):
            pools = (xpool, wpool, hpool, spool, opool, psu, psg, psz)
            # PE body > 256 instructions: hint the back-edge so the loop
            # branch I$-hits instead of stalling ~4us on an IRAM fetch
            loop_cm = (tc.For_i(0, reps, 1, hint_engines=(mybir.EngineType.PE,))
                       if reps > 1 else contextlib.nullcontext())
            with loop_cm:
                _emit_body(nc, blocks, pools)
    nc.finalize()
    return nc


def _emit_body(nc, blocks, pools):
    xpool, wpool, hpool, spool, opool, psu, psg, psz = pools
    last = len(blocks) - 1
    for bi, (w1, w2, NHb, xt, C, zt, wdt, ascale, xdt, dr) in enumerate(blocks):
        w1_a = w1.ap()
        w2_a = w2.ap()
        xt_a = xt.ap().rearrange("p (k c) -> p k c", k=KD)
        zt_a = zt.ap().rearrange("p (m c) -> p m c", m=MD)

        x_tile = xpool.tile([P, KD, C], xdt, tag=f"x{bi}")
        nc.scalar.dma_start(x_tile[:], xt_a)
        h_tile = hpool.tile([P, NHb, C], F16, tag=f"h{bi}")

        # fc1 weight chunks, in consumption order on the sync FIFO
        csz1 = GH * 2 * KD * P
        w1ts = []
        for cc in range(NHb // GH):
            if dr:
                w1t = wpool.tile([P, GH, 2, KD // 2, 2, P], wdt, tag=f"w1_{bi}_{cc}")
                w1t_src = w1_a[:, cc * csz1:(cc + 1) * csz1].rearrange(
                    "p (h s j t f) -> p h s j t f", h=GH, s=2, j=KD // 2, t=2)
            else:
                w1t = wpool.tile([P, GH, 2, KD, P], wdt, tag=f"w1_{bi}_{cc}")
                w1t_src = w1_a[:, cc * csz1:(cc + 1) * csz1].rearrange(
                    "p (h s k f) -> p h s k f", h=GH, s=2, k=KD)
            nc.sync.dma_start(w1t[:], w1t_src)
            w1ts.append(w1t)

        for hc in range(NHb):
            w1t = w1ts[hc // GH]
            g = hc % GH
            for c0 in range(0, C, 512):
                c1 = min(C, c0 + 512)
                ps_u = psu.tile([P, c1 - c0], F32, tag="psu")
                ps_g = psg.tile([P, c1 - c0], F32, tag="psg")
                if dr:
                    for j in range(KD // 2):
                        nc.tensor.matmul(ps_u[:], w1t[:, g, 0, j],
                                         x_tile[:, 2 * j:2 * j + 2, c0:c1],
                                         start=(j == 0), stop=(j == KD // 2 - 1),
                                         perf_mode=DROW)
                    for j in range(KD // 2):
                        nc.tensor.matmul(ps_g[:], w1t[:, g, 1, j],
                                         x_tile[:, 2 * j:2 * j + 2, c0:c1],
                                         start=(j == 0), stop=(j == KD // 2 - 1),
                                         perf_mode=DROW)
                else:
                    for k in range(KD):
                        nc.tensor.matmul(ps_u[:], w1t[:, g, 0, k], x_tile[:, k, c0:c1],
                                         start=(k == 0), stop=(k == KD - 1))
                    for k in range(KD):
                        nc.tensor.matmul(ps_g[:], w1t[:, g, 1, k], x_tile[:, k, c0:c1],
                                         start=(k == 0), stop=(k == KD - 1))
                sil = spool.tile([P, c1 - c0], F32, tag="sil")
                if ascale != 1.0:
                    nc.scalar.activation(sil[:], ps_g[:], ACT.Silu, scale=ascale)
                else:
                    nc.scalar.activation(sil[:], ps_g[:], ACT.Silu)
                nc.vector.tensor_mul(h_tile[:, hc, c0:c1], ps_u[:], sil[:])

        o_tile = opool.tile([P, MD, C], F16, tag=f"o{bi}")
        csz2 = GD * NHb * P
        for cc in range(MD // GD):
            w2t = wpool.tile([P, GD, NHb, P], wdt, tag=f"w2_{bi}_{cc}")
            nc.sync.dma_start(
                w2t[:],
                w2_a[:, cc * csz2:(cc + 1) * csz2].rearrange(
                    "p (d k f) -> p d k f", d=GD, k=NHb))
            for d in range(GD):
                for c0 in range(0, C, 512):
                    c1 = min(C, c0 + 512)
                    ps_z = psz.tile([P, c1 - c0], F32, tag="psz")
                    for k in range(NHb):
                        nc.tensor.matmul(ps_z[:], w2t[:, d, k], h_tile[:, k, c0:c1],
                                         start=(k == 0), stop=(k == NHb - 1))
                    nc.vector.tensor_copy(o_tile[:, GD * cc + d, c0:c1], ps_z[:])
            if bi == last:
                # split the final store so most of it overlaps compute
                nc.scalar.dma_start(zt_a[:, GD * cc:GD * (cc + 1)],
                                    o_tile[:, GD * cc:GD * (cc + 1)])
        if bi != last:
            nc.scalar.dma_start(zt_a, o_tile[:])


def _route(xf, Wg):
    """Host gate: softmax over expert logits, top-2 (ties -> lower index,
    matching jax.lax.top_k)."""
    logits = xf @ Wg.T
    m = logits.max(axis=-1, keepdims=True)
    p = np.exp(logits - m)
    scores = p / p.sum(axis=-1, keepdims=True)
    i1 = scores.argmax(axis=-1)
    rows = np.arange(T)
    s1 = scores[rows, i1]
    masked = scores.copy()
    masked[rows, i1] = -np.inf
    i2 = masked.argmax(axis=-1)
    s2 = scores[rows, i2]
    return i1, s1 * ROUTE_SCALE, i2, s2 * ROUTE_SCALE


def _pack_w1(W1b, HB, npt, scale=1.0):
    """[D, 2*HB] -> [P, NHb*2*KD*P] flat per partition, (hc, half, ko, f).
    (For DoubleRow the ko axis is viewed as (ko//2, ko%2) — same layout.)"""
    NHb = HB // P
    Ar = (W1b * scale).reshape(KD, P, 2, NHb, P)   # [ko, ki, half, hc, f]
    return np.ascontiguousarray(
        Ar.transpose(1, 3, 2, 0, 4).reshape(P, NHb * 2 * KD * P).astype(npt))


def _pack_w2(W2b, HB, npt, scale=1.0):
    """[HB, D] -> [P, MD*NHb*P] flat per partition, (dc, ko, f)."""
    NHb = HB // P
    Br = (W2b * scale).reshape(NHb, P, MD, P)      # [ko, ki, dc, f]
    return np.ascontiguousarray(
        Br.transpose(1, 2, 0, 3).reshape(P, MD * NHb * P).astype(npt))


def _pack_x(xTb, C, npt=np.float16):
    """[D, n] -> [P, KD*C] (zero-pads the token dim to C)."""
    n = xTb.shape[1]
    out = np.zeros((P, KD, C), dtype=npt)
    out[:, :, :n] = xTb.reshape(KD, P, n).transpose(1, 0, 2).astype(npt)
    return out.reshape(P, KD * C)


def _r32(n):
    return max(32, -(-n // 32) * 32)


def prepare(x, Wg, W1, W2, Ws1, Ws2):
    """Host routing + per-core input maps.

    Returns (in_maps, toks, wts, assign, CA, CB) where assign[c] is the
    (expertA, expertB) pair owned by core c."""
    x = np.asarray(x, dtype=np.float32)
    Wg = np.asarray(Wg, dtype=np.float32)
    W1 = np.asarray(W1, dtype=np.float32)
    W2 = np.asarray(W2, dtype=np.float32)
    Ws1 = np.asarray(Ws1, dtype=np.float32)
    Ws2 = np.asarray(Ws2, dtype=np.float32)

    xf = np.ascontiguousarray(x.reshape(T, D))
    i1, s1, i2, s2 = _route(xf, Wg)

    toks, wts = [], []
    for e in range(E):
        sel = np.where((i1 == e) | (i2 == e))[0]
        toks.append(sel)
        wts.append(np.where(i1[sel] == e, s1[sel], s2[sel]).astype(np.float32))

    # Pair the 8 largest with the 8 smallest: slot A holds the big ones.
    order = sorted(range(E), key=lambda e: -len(toks[e]))
    assign = [(order[c], order[E - 1 - c]) for c in range(N_CORES)]
    CA = _r32(max(len(toks[a]) for a, _ in assign))
    CB = _r32(max(len(toks[b]) for _, b in assign))

    if ROUTED_FP8:
        npt_r, wscale = NP8, W8SCALE
    else:
        npt_r, wscale = NP16, 1.0
    npt_x = NP8 if (ROUTED_FP8 and DR_FC1) else NP16

    # Shared expert, split by hidden half: q=0 -> cols [0,HL), q=1 -> [HL,HS)
    ws1p, ws2p = [], []
    for q in range(2):
        W1s = np.concatenate(
            [Ws1[:, q * HL:(q + 1) * HL], Ws1[:, HS + q * HL:HS + (q + 1) * HL]],
            axis=1)
        ws1p.append(_pack_w1(W1s, HL, NP16))
        ws2p.append(_pack_w2(Ws2[q * HL:(q + 1) * HL], HL, NP16))

    in_maps = []
    for c in range(N_CORES):
        grp, q = c // 2, c % 2
        xd = _pack_x(np.ascontiguousarray(
            xf[grp * TDP2:(grp + 1) * TDP2].T), TDP2)
        im = {"ws1": ws1p[q], "ws2": ws2p[q], "xd": xd}
        for i, (e, cap) in enumerate(zip(assign[c], (CA, CB))):
            im[f"w1e{i}"] = _pack_w1(W1[e], H, npt_r, wscale)
            im[f"w2e{i}"] = _pack_w2(W2[e], H, npt_r, wscale)
            im[f"xg{i}"] = _pack_x(xf[toks[e]].T, cap, npt_x)
        in_maps.append(im)
    return in_maps, toks, wts, assign, CA, CB


def kernel(x, Wg, W1, W2, Ws1, Ws2):
    global LAST_RESULTS
    in_maps, toks, wts, assign, CA, CB = prepare(x, Wg, W1, W2, Ws1, Ws2)

    key = (CA, CB)
    if key not in _NC_CACHE:
        _NC_CACHE[key] = _build_nc(CA, CB)
    nc = _NC_CACHE[key]

    try:
        LAST_RESULTS = run_bass_kernel_spmd(nc, in_maps, list(range(N_CORES)))
    except Exception:
        # transient NRT device errors have been observed; retry once
        LAST_RESULTS = run_bass_kernel_spmd(nc, in_maps, list(range(N_CORES)))
    res = LAST_RESULTS.results

    zscale = 1.0 / (W8SCALE * W8SCALE) if ROUTED_FP8 else 1.0
    out = np.zeros((T, D), dtype=np.float32)
    for c in range(N_CORES):
        grp = c // 2
        for i, (e, cap) in enumerate(zip(assign[c], (CA, CB))):
            n = len(toks[e])
            z = res[c][f"zg{i}"].astype(np.float32).reshape(P, MD, cap)
            z = z.transpose(1, 0, 2).reshape(D, cap)[:, :n]
            out[toks[e]] += (zscale * wts[e])[:, None] * z.T
        zs = res[c]["zs"].astype(np.float32).reshape(P, MD, TDP2)
        out[grp * TDP2:(grp + 1) * TDP2] += zs.transpose(1, 0, 2).reshape(D, TDP2).T
    return out.reshape(B, S, D)
